# revision 34
# baseline (speedup 1.0000x reference)
"""Trainium2 Bass kernel for nn_NeuralMemory (test-time-training memory layer).

Mathematical reformulation (validated vs the jax reference):
  * Every per-chunk gradient is taken at the same initial params, and the
    two chunk-axis linear scans (momentum, decay) are linear in the
    gradients.  The final updated weights collapse to a single
    token-weighted backward pass with per-token weight
        rho_t = (2/d) * W_{c(t)} * lr_{c(t)},
        W_j   = K_j + eta_{j+1} W_{j+1},  K_j = prod_{i>j} keep_i.
    rho is computed on the HOST and shipped as a [128, 32] per-token-tile
    column table; the device never evaluates sigmoids or scans.
  * gamma0 is ones (spec fill), so the forward residual-norm scale drops
    out of the backward: with w := h*ri + (k-v),  rr := ri*rho,
        t3  = w*rr,   q1 = t3 .* h,   cp' = sum_f q1 = rr*cp
        s2  = -cp'*ri^2/D,   dh = h*s2 + t3
        dg  = sum_tok q1      (via PE transpose + DVE row-accum)
        dw1 = wk^T M,  M = sum_t S_t^T dz_t   (dw1 never materialized:
              retrieval uses w1c = -(wq wk^T) M with KT = wk wq^T host-made)
        dw2T += dh^T a
  * z is computed directly from seq via the host-folded w1k = wk @ w1_0
    in BOTH layouts (token-major pair-wide for a/gelu'(z), hidden-major
    pair-wide for the h matmul): no k projection/copy on device at all.
  * The per-token backward scalars (rr, rir, s3 with
    cp = ri*ss + cv, cv = sum_f kv.*h accumulated in fwd) are computed
    GROUP-BATCHED in the rsqrt step, so the per-tile critical chain is
    just kvr(Pool) -> dh = h*s3 + kvr (one DVE stt) -> dhT -> da -> dz
    -> M; the q1/dg path trails off-critical on Pool.
  * Retrieval z comes from seqrT via w1c (device-folded wq @ w1f); h/qt
    are evacuated from PSUM immediately (ACT bf16 / DVE fp32 copies) so
    the 2-slot PSUM rotation never gates the group pipeline.

Engine assignment targets balanced busy-time: wide [128,1024] 2-bank
PSUM tiles let one ACT instruction cover a token-tile PAIR (fixed
~185ns ACT overhead amortized); matmul cost scales with OUTPUT width
and PE p-states reward continuous streaks, so fwd emits pure PE/ACT
work and all DVE/Pool consumers (stats, bwd chains) are emitted where
they cannot poison another engine's in-order queue.  GPSIMD (Pool)
cannot touch PSUM, so it only gets SBUF elementwise work.

Sharding: core c handles sample s=c//2; grad replicated in the core
pair, retrieval split by half (no collectives -- a 2-core AllReduce
costs ~15us kickoff on this stack, more than the ~8us it would save).
"""
import numpy as np
import ml_dtypes

import concourse.bass as bass
import concourse.bacc as bacc
import concourse.mybir as mybir
import concourse.tile as tile
from concourse import bass_utils

FP = mybir.dt.float32
BF = mybir.dt.bfloat16
AF = mybir.ActivationFunctionType
OP = mybir.AluOpType

B, N, D, HID = 4, 4096, 128, 512
CHUNK = 64
NC = N // CHUNK            # 64 chunks
NT = N // 128              # 32 token-tiles (grad chain)
NRT = (N // 2) // 128      # 16 token-tiles (retrieval half)
QT = 4                     # grad tiles per group (rsqrt batching)
RQ = 4                     # retrieval tiles per group
NCORES = 8

_CACHED = {}

# cpb (bf16) column offsets
_W1K = 0           # wk @ w1_0, 512 cols
_WKV = 512         # wk - wv, 128
_W2C = 640         # w2 block layout [hid_c, (c,feat)], 512
_W2T = 1152        # w2^T [feat, hid], 512
_IDTB = 1664       # identity bf16, 128
_KT = 1792         # wk @ wq^T (for w1c = -(wq wk^T) M), 128
_WQ = 1920         # wq, 128
_CPB_COLS = 2048

# cpf (fp32) column offsets
_IDT = 0           # identity fp32
_RHO = 128         # rho_tok [128, 32]
_CPF_COLS = 160


def _emit_rsqrt(nc, wpool, ss, scale, bias, w, tagp):
    """ri = 1/sqrt(ss*scale + bias) on DVE only (Quake init + 1 Newton)."""
    I32 = mybir.dt.int32
    ms = wpool.tile([128, w], FP, tag=f"{tagp}q_ms")
    nc.vector.tensor_scalar(ms[:], ss, scale, bias, OP.mult, OP.add)
    qi = wpool.tile([128, w], I32, tag=f"{tagp}q_i")
    nc.vector.tensor_scalar(qi[:], ms[:].bitcast(I32), 1, None,
                            OP.arith_shift_right)
    qj = wpool.tile([128, w], I32, tag=f"{tagp}q_j")
    nc.vector.tensor_scalar(qj[:], qi[:], -1, 0x5F3759DF, OP.mult, OP.add)
    y = qj[:].bitcast(FP)
    a = wpool.tile([128, w], FP, tag=f"{tagp}q_a")
    nc.vector.tensor_mul(a[:], y, y)
    nc.vector.tensor_mul(a[:], a[:], ms[:])
    nc.vector.tensor_scalar(a[:], a[:], -0.5, 1.5, OP.mult, OP.add)
    yn = wpool.tile([128, w], FP, tag=f"{tagp}q_y")
    nc.vector.tensor_mul(yn[:], y, a[:])
    return yn[:]


def build_nc(repeat=1, nkeep=NT, das_pairs=None, phase="all"):
    nc = bacc.Bacc("TRN2", target_bir_lowering=False, debug=False)

    T0 = NT - nkeep
    W = nkeep * 128
    NP = nkeep // 2            # grad pairs
    NG = nkeep // QT           # grad groups
    if das_pairs is None:
        # das-route dz2 through ACT everywhere: ACT absorbs the psum
        # evacuation and DVE gets the cheap 2x SBUF multiply
        das_pairs = tuple(range(NP))

    # ---- DRAM I/O ----
    seqT_d = nc.dram_tensor("seqT", [D, W], BF, kind="ExternalInput")
    seqG_d = nc.dram_tensor("seqG", [128, W], BF, kind="ExternalInput")
    seqrT_d = nc.dram_tensor("seqrT", [D, N // 2], BF, kind="ExternalInput")
    cpb_d = nc.dram_tensor("cpb", [128, _CPB_COLS], BF, kind="ExternalInput")
    cpf_d = nc.dram_tensor("cpf", [128, _CPF_COLS], FP, kind="ExternalInput")
    out_d = nc.dram_tensor("out", [N // 2, D], FP, kind="ExternalOutput")

    with tile.TileContext(nc) as tc:
        with (
            tc.tile_pool(name="const", bufs=1) as cpool,
            tc.tile_pool(name="seq", bufs=1) as spool,
            tc.tile_pool(name="fin", bufs=2) as fpool,
            tc.tile_pool(name="work", bufs=4) as wpool,
            tc.tile_pool(name="qcol", bufs=3) as qpool,
            tc.tile_pool(name="pacc", bufs=1, space="PSUM") as pacc,
            tc.tile_pool(name="pwide", bufs=2, space="PSUM") as pwide,
            tc.tile_pool(name="pmid", bufs=2, space="PSUM") as pmid,
        ):
            # ---- constants & sequence into SBUF: separate tiles per DMA
            # chunk so consumers only wait on their own transfer ----
            cpbA = cpool.tile([128, 640], BF)    # w1k | wkv
            cpbB = cpool.tile([128, 1152], BF)   # w2c | w2T | IDTb
            cpbC = cpool.tile([128, 256], BF)    # KT | wq
            cpf = cpool.tile([128, _CPF_COLS], FP)
            seqTs = [spool.tile([D, 256], BF, name=f"seqT{p}")
                     for p in range(NP)]
            seqGs = [spool.tile([128, 512], BF, name=f"seqG{g}")
                     for g in range(NG)]
            seqrT = spool.tile([D, N // 2], BF)

            nc.sync.dma_start(cpbA[:], cpb_d.ap()[:, 0:640])
            nc.sync.dma_start(seqTs[0][:], seqT_d.ap()[:, 0:256])
            nc.sync.dma_start(cpbB[:], cpb_d.ap()[:, 640:1792])
            nc.sync.dma_start(cpf[:], cpf_d.ap())
            for p in range(1, NP):
                nc.sync.dma_start(seqTs[p][:],
                                  seqT_d.ap()[:, p * 256:(p + 1) * 256])
            for g in range(NG):
                nc.sync.dma_start(seqGs[g][:],
                                  seqG_d.ap()[:, g * 512:(g + 1) * 512])
            nc.sync.dma_start(cpbC[:], cpb_d.ap()[:, 1792:2048])
            for j in range(2):
                nc.sync.dma_start(seqrT[:, j * 1024:(j + 1) * 1024],
                                  seqrT_d.ap()[:, j * 1024:(j + 1) * 1024])

            w1k = cpbA[:, 0:512]
            wkv = cpbA[:, 512:640]
            w2c = cpbB[:, 0:512]
            w2T = cpbB[:, 512:1024]
            IDTb = cpbB[:, 1024:1152]
            KT = cpbC[:, 0:128]
            wq = cpbC[:, 128:256]
            IDT = cpf[:, _IDT:_IDT + 128]
            rho = cpf[:, _RHO:_RHO + NT]

            def seqT_pair(p):
                return seqTs[p][:]

            def seqT_tile(p, j):
                return seqTs[p][:, j * 128:(j + 1) * 128]

            for _rep in range(repeat):
                # =========================================================
                # Gradient chain over the kept suffix: software-pipelined
                # groups of QT tiles; M/dw2T accumulated in PSUM.
                # =========================================================
                M_acc = pacc.tile([D, HID], FP, tag="M")
                W2_acc = pacc.tile([128, HID], FP, tag="W2")
                dgparts = fpool.tile([128, NP], FP, tag="dgparts")

                pairs = {}      # p -> sbuf tiles from fwd
                quads = {}      # q -> ss4 or batched scalar columns
                cv4s = {}       # q -> cv accumulator [128, QT]
                dz2s = {}       # p -> dz2 tile (for deferred M matmuls)

                def grad_fwd_pair(p):
                    t0 = 2 * p
                    q, j0 = divmod(t0, QT)
                    if j0 == 0:
                        quads[q] = qpool.tile([128, QT], FP, tag="ss4",
                                              name="ss4")
                        cv4s[q] = qpool.tile([128, QT], FP, tag="cv4",
                                             name="cv4")

                    # hidden-major z pair first: afab -> mix -> h2kv is the
                    # chain gating the group's rsq, so it leads ACT's queue
                    zf = pwide.tile([128, 1024], FP, tag="w", name="zf")
                    for c in range(4):
                        nc.tensor.matmul(zf[:, c * 256:(c + 1) * 256],
                                         w1k[:, c * 128:(c + 1) * 128],
                                         seqT_pair(p), start=True,
                                         stop=True)
                    afab = wpool.tile([128, 1024], BF, tag="afab", bufs=3)
                    nc.scalar.activation(afab[:], zf[:], AF.Gelu)

                    # h(t0)|h(t1)|kv(t0)|kv(t1) in one bank
                    mix = pmid.tile([128, 512], FP, tag="m", name="mix")
                    for j in range(2):
                        for c in range(4):
                            nc.tensor.matmul(
                                mix[:, j * 128:(j + 1) * 128],
                                afab[:, c * 256 + j * 128:
                                     c * 256 + (j + 1) * 128],
                                w2c[:, c * 128:(c + 1) * 128],
                                start=(c == 0), stop=(c == 3))
                        nc.tensor.matmul(mix[:, 256 + j * 128:
                                             256 + (j + 1) * 128],
                                         seqT_tile(p, j),
                                         wkv, start=True, stop=True)
                    # single pair-batched PSUM->SBUF copy (ACT): h | kv
                    h2kv = wpool.tile([128, 512], BF, tag="h2kv", bufs=6)
                    nc.scalar.copy(h2kv[:], mix[:])
                    # token-major z pair (one wide 2-bank psum tile)
                    zp = pwide.tile([128, 1024], FP, tag="w", name="zp")
                    for j in range(2):
                        nc.tensor.matmul(zp[:, j * 512:(j + 1) * 512],
                                         seqT_tile(p, j),
                                         w1k, start=True, stop=True)
                    a_tm2 = wpool.tile([128, 1024], BF, tag="a_tm2", bufs=4)
                    nc.scalar.activation(a_tm2[:], zp[:], AF.Gelu)
                    gp2 = wpool.tile([128, 1024], BF, tag="gp2", bufs=4)
                    nc.scalar.activation(gp2[:], zp[:], AF.Derivative_Gelu)
                    pairs[p] = (h2kv, a_tm2, gp2)

                def grad_stats_pair(p):
                    # ss/cv accumulation, off the critical bwd chain
                    t0 = 2 * p
                    q, j0 = divmod(t0, QT)
                    h2kv, a_tm2, gp2 = pairs[p]
                    scr = wpool.tile([128, 256], BF, tag="scr", bufs=6)
                    nc.gpsimd.tensor_mul(scr[:], h2kv[:, 0:256],
                                         h2kv[:, 0:256])
                    sdead = wpool.tile([128, 128], BF, tag="sdead", bufs=4)
                    cvscr = wpool.tile([128, 128], BF, tag="cvscr", bufs=4)
                    for j in range(2):
                        nc.vector.tensor_scalar(
                            sdead[:], scr[:, j * 128:(j + 1) * 128],
                            1.0, 0.0, OP.mult, OP.add,
                            accum_out=quads[q][:, j0 + j:j0 + j + 1])
                        nc.vector.scalar_tensor_tensor(
                            cvscr[:], h2kv[:, 256 + j * 128:
                                           256 + (j + 1) * 128], 1.0,
                            h2kv[:, j * 128:(j + 1) * 128], OP.mult, OP.mult,
                            accum_out=cv4s[q][:, j0 + j:j0 + j + 1])

                def grad_rsq(q):
                    ss4 = quads[q]
                    cv4 = cv4s[q]
                    ri4 = _emit_rsqrt(nc, qpool, ss4[:], 1.0 / D, 1e-6,
                                      QT, "g")
                    rr4 = qpool.tile([128, QT], FP, tag="rr4")
                    nc.vector.tensor_mul(rr4[:], ri4,
                                         rho[:, T0 + QT * q:
                                              T0 + QT * q + QT])
                    rir = qpool.tile([128, QT], FP, tag="rir")
                    nc.vector.tensor_mul(rir[:], ri4, rr4[:])
                    # s3 = rir + s2,  s2 = -(rr*ri^2/D) * (ri*ss + cv)
                    e2 = qpool.tile([128, QT], FP, tag="e2")
                    nc.vector.tensor_mul(e2[:], ri4, ss4[:])
                    nc.vector.tensor_add(e2[:], e2[:], cv4[:])
                    f1 = qpool.tile([128, QT], FP, tag="f1")
                    nc.vector.tensor_mul(f1[:], ri4, ri4)
                    nc.vector.tensor_mul(f1[:], f1[:], rr4[:])
                    nc.vector.tensor_mul(f1[:], f1[:], e2[:])
                    s3 = qpool.tile([128, QT], FP, tag="s3")
                    nc.vector.scalar_tensor_tensor(s3[:], f1[:], -1.0 / D,
                                                   rir[:], OP.mult, OP.add)
                    quads[q] = (rr4, rir, s3)

                def grad_bwd_pair(p):
                    t0 = 2 * p
                    q = t0 // QT
                    rr4, rir, s3 = quads[q]
                    h2kv, a_tm2, gp2 = pairs.pop(p)
                    da2 = pwide.tile([128, 1024], FP, tag="w", name="da2")
                    tr = pmid.tile([128, 512], BF, tag="m", name="tr")
                    cols = [(rr4[:, (t0 + j) % QT:(t0 + j) % QT + 1],
                             rir[:, (t0 + j) % QT:(t0 + j) % QT + 1],
                             s3[:, (t0 + j) % QT:(t0 + j) % QT + 1],
                             h2kv[:, j * 128:(j + 1) * 128],
                             h2kv[:, 256 + j * 128:256 + (j + 1) * 128])
                            for j in range(2)]
                    # critical chain for BOTH tiles first:
                    # dh = h*s3 + kv*rr -> dhT -> da -> dz -> M
                    kvrs, dhs = [], []
                    for j, (rr_c, rir_c, s3_c, h_sb, kv) in enumerate(cols):
                        kvr = wpool.tile([128, 128], BF, tag="kvr", bufs=8)
                        nc.gpsimd.tensor_scalar_mul(kvr[:], kv, rr_c)
                        kvrs.append(kvr)
                    for j, (rr_c, rir_c, s3_c, h_sb, kv) in enumerate(cols):
                        dh = wpool.tile([128, 128], BF, tag="dh", bufs=8)
                        nc.vector.scalar_tensor_tensor(dh[:], h_sb, s3_c,
                                                       kvrs[j][:], OP.mult,
                                                       OP.add)
                        dhs.append(dh)
                        nc.tensor.transpose(tr[:, j * 128:(j + 1) * 128],
                                            dh[:], IDTb)
                    dhT2 = wpool.tile([128, 256], BF, tag="dhT2", bufs=6)
                    nc.vector.tensor_copy(dhT2[:], tr[:, 0:256])
                    for j in range(2):
                        nc.tensor.matmul(da2[:, j * 512:(j + 1) * 512],
                                         dhT2[:, j * 128:(j + 1) * 128],
                                         w2T, start=True, stop=True)
                    dz2 = wpool.tile([128, 1024], BF, tag="dz2", bufs=3)
                    if p in das_pairs:
                        das2 = wpool.tile([128, 1024], BF, tag="das2",
                                          bufs=3)
                        nc.scalar.copy(das2[:], da2[:])
                        nc.vector.tensor_mul(dz2[:], das2[:], gp2[:])
                    else:
                        nc.vector.tensor_mul(dz2[:], da2[:], gp2[:])
                    dz2s[p] = dz2
                    # off-critical trail: W2 accumulation, dg path
                    for j, (rr_c, rir_c, s3_c, h_sb, kv) in enumerate(cols):
                        t = t0 + j
                        nc.tensor.matmul(W2_acc[:], dhs[j][:],
                                         a_tm2[:, j * 512:(j + 1) * 512],
                                         start=(t == 0), stop=(t == nkeep - 1))
                        u1 = wpool.tile([128, 128], BF, tag="u1", bufs=8)
                        nc.gpsimd.tensor_scalar_mul(u1[:], h_sb, rir_c)
                        t3 = wpool.tile([128, 128], BF, tag="t3", bufs=8)
                        nc.gpsimd.tensor_add(t3[:], u1[:], kvrs[j][:])
                        q1 = wpool.tile([128, 128], BF, tag="q1", bufs=8)
                        nc.gpsimd.tensor_mul(q1[:], t3[:], h_sb)
                        nc.tensor.transpose(tr[:, 256 + j * 128:
                                               256 + (j + 1) * 128],
                                            q1[:], IDTb)
                    gdead = wpool.tile([128, 256], BF, tag="gdead", bufs=4)
                    nc.vector.tensor_scalar(
                        gdead[:], tr[:, 256:512],
                        1.0, 0.0, OP.mult, OP.add,
                        accum_out=dgparts[:, p:p + 1])

                def grad_m_pair(p):
                    # deferred M matmuls (wait on the slow dz2 DVE op, so
                    # emitted after the next fwd's PE work)
                    t0 = 2 * p
                    dz2 = dz2s.pop(p)
                    for j in range(2):
                        t = t0 + j
                        nc.tensor.matmul(M_acc[:],
                                         seqGs[p // 2][:, (t0 % 4 + j) * 128:
                                                       (t0 % 4 + j + 1) * 128],
                                         dz2[:, j * 512:(j + 1) * 512],
                                         start=(t == 0), stop=(t == nkeep - 1))

                # software pipeline: fwd(q) feeds ACT/PE with ready work
                # (no DVE/Pool ops inside), bwd(q-1) critical chains run
                # on DVE/Pool behind it, deferred M and stats last
                for q in range(NG + 1):
                    for pj in range(QT // 2):
                        if q < NG:
                            grad_fwd_pair((QT * q) // 2 + pj)
                        if q >= 1:
                            grad_bwd_pair((QT * (q - 1)) // 2 + pj)
                        if q < NG:
                            grad_stats_pair((QT * q) // 2 + pj)
                    if q < NG:
                        grad_rsq(q)
                    if q >= 1:
                        for pj in range(QT // 2):
                            grad_m_pair((QT * (q - 1)) // 2 + pj)

                # =========================================================
                # Bridge: final weights for retrieval
                #   w1c = -(wq wk^T) M,  w2f = -(dw2T)^T,  gfb = -dg bcast
                # =========================================================
                Msb = fpool.tile([D, HID], BF, tag="Msb")
                nc.scalar.copy(Msb[:], M_acc[:])
                w2Tf = fpool.tile([128, HID], BF, tag="w2Tf")
                nc.scalar.activation(w2Tf[:], W2_acc[:], AF.Copy, scale=-1.0)

                ps_w1c = pwide.tile([128, 512], FP, tag="w", name="ps_w1c")
                nc.tensor.matmul(ps_w1c[:], KT, Msb[:], start=True, stop=True)
                w1c = fpool.tile([D, HID], BF, tag="w1c")
                nc.scalar.activation(w1c[:], ps_w1c[:], AF.Copy, scale=-1.0)

                ps_w2 = pmid.tile([128, 512], BF, tag="m", name="ps_w2")
                for c in range(4):
                    nc.tensor.transpose(ps_w2[:, c * 128:(c + 1) * 128],
                                        w2Tf[:, c * 128:(c + 1) * 128], IDTb)
                w2f = fpool.tile([128, HID], BF, tag="w2f")
                nc.vector.tensor_copy(w2f[:], ps_w2[:])

                # gf = -sum_p dgparts ; gfb2 = broadcast row, twice
                gfc = fpool.tile([128, 1], FP, tag="gfc")
                gdead2 = fpool.tile([128, NP], FP, tag="gdead2")
                nc.vector.tensor_scalar(gdead2[:], dgparts[:], -1.0, 0.0,
                                        OP.mult, OP.add, accum_out=gfc[:])
                ps_gT = pmid.tile([1, 128], FP, tag="m", name="ps_gT")
                nc.tensor.transpose(ps_gT[:], gfc[:], IDT)
                gT = fpool.tile([1, 128], FP, tag="gT")
                nc.scalar.copy(gT[:], ps_gT[:])
                ones_r = fpool.tile([1, 128], FP, tag="ones_r")
                nc.vector.memset(ones_r[:], 1.0)
                ps_gfb = pwide.tile([128, 128], FP, tag="w", name="ps_gfb")
                nc.tensor.matmul(ps_gfb[:], ones_r[:], gT[:],
                                 start=True, stop=True)
                gfb2 = fpool.tile([128, 256], BF, tag="gfb2")
                nc.vector.tensor_copy(gfb2[:, 0:128], ps_gfb[:])
                nc.vector.tensor_copy(gfb2[:, 128:256], gfb2[:, 0:128])

                if phase == "grad":
                    # debug: dump bridge results and skip retrieval
                    dbg = fpool.tile([128, 256], FP, tag="dbg")
                    nc.vector.tensor_copy(dbg[:, 0:4], dgparts[:])
                    nc.vector.tensor_copy(dbg[:, 4:5], gfc[:])
                    nc.sync.dma_start(out_d.ap()[0:128, 0:128],
                                      dbg[:, 0:128])
                    continue

                # =========================================================
                # Retrieval on this core's half (16 tiles, pipelined)
                # =========================================================
                rpairs = {}
                rquads = {}
                opacks = {}

                def ret_fwd_pair(r):
                    i0 = 2 * r
                    g, j0 = divmod(i0, RQ)
                    if j0 == 0:
                        rquads[g] = qpool.tile([128, RQ], FP, tag="ss2",
                                               name="ss2")
                    ss2 = rquads[g]
                    Sr2 = seqrT[:, i0 * 128:(i0 + 2) * 128]

                    z2 = pwide.tile([128, 1024], FP, tag="w", name="z2")
                    for c in range(4):
                        nc.tensor.matmul(z2[:, c * 256:(c + 1) * 256],
                                         w1c[:, c * 128:(c + 1) * 128],
                                         Sr2, start=True, stop=True)
                    a2 = wpool.tile([128, 1024], BF, tag="a2", bufs=3)
                    nc.scalar.activation(a2[:], z2[:], AF.Gelu)

                    hq = pmid.tile([128, 512], FP, tag="m", name="hq")
                    for j in range(2):
                        for c in range(4):
                            nc.tensor.matmul(
                                hq[:, j * 128:(j + 1) * 128],
                                a2[:, c * 256 + j * 128:
                                   c * 256 + (j + 1) * 128],
                                w2f[:, c * 128:(c + 1) * 128],
                                start=(c == 0), stop=(c == 3))
                        nc.tensor.matmul(hq[:, 256 + j * 128:
                                             256 + (j + 1) * 128],
                                         seqrT[:, (i0 + j) * 128:
                                               (i0 + j + 1) * 128],
                                         wq, start=True, stop=True)
                    # evacuate PSUM immediately: h2 (bf16, ACT), qt (fp32,
                    # DVE); ss via Pool square + DVE row-accum from SBUF
                    h2r = wpool.tile([128, 256], BF, tag="h2r", bufs=6)
                    nc.scalar.copy(h2r[:], hq[:, 0:256])
                    qtr = wpool.tile([128, 256], FP, tag="qtr", bufs=6)
                    nc.vector.tensor_copy(qtr[:], hq[:, 256:512])
                    scr2 = wpool.tile([128, 256], BF, tag="scr2", bufs=6)
                    nc.gpsimd.tensor_mul(scr2[:], h2r[:], h2r[:])
                    sdead2 = wpool.tile([128, 128], BF, tag="sdead2", bufs=4)
                    for j in range(2):
                        nc.vector.tensor_scalar(
                            sdead2[:], scr2[:, j * 128:(j + 1) * 128],
                            1.0, 0.0, OP.mult, OP.add,
                            accum_out=ss2[:, j0 + j:j0 + j + 1])
                    rpairs[r] = (h2r, qtr)

                def ret_rsq(g):
                    r2i = _emit_rsqrt(nc, qpool, rquads[g][:], 1.0 / D,
                                      1e-6, RQ, "r")
                    rquads[g] = r2i

                def ret_out_pair(r):
                    i0 = 2 * r
                    g = i0 // RQ
                    r2i = rquads[g]
                    h2r, qtr = rpairs.pop(r)
                    hn2p = wpool.tile([128, 256], BF, tag="hn2p", bufs=6)
                    for j in range(2):
                        jj = (i0 + j) % RQ
                        nc.vector.tensor_scalar(
                            hn2p[:, j * 128:(j + 1) * 128],
                            h2r[:, j * 128:(j + 1) * 128],
                            r2i[:, jj:jj + 1], None, OP.mult)
                    o1p = wpool.tile([128, 256], BF, tag="o1p", bufs=6)
                    nc.vector.tensor_mul(o1p[:], hn2p[:], gfb2[:])
                    if r % 2 == 0:
                        opacks[g] = wpool.tile([128, 512], FP, tag="opack",
                                               name="opack", bufs=3)
                    opack = opacks[g]
                    off = (r % 2) * 256
                    nc.gpsimd.tensor_add(opack[:, off:off + 256], o1p[:],
                                         qtr[:])
                    if r % 2 == 1:
                        dst = out_d.ap()[g * 512:(g + 1) * 512,
                                         :].rearrange("(j p) d -> p j d",
                                                      p=128)
                        nc.sync.dma_start(
                            dst, opack[:].rearrange("p (j d) -> p j d",
                                                    d=128))

                for g in range(NRT // RQ + 1):
                    for rj in range(RQ // 2):
                        if g < NRT // RQ:
                            ret_fwd_pair((RQ * g) // 2 + rj)
                        if g >= 1 and phase != "ret_fwd":
                            ret_out_pair((RQ * (g - 1)) // 2 + rj)
                    if g < NRT // RQ:
                        ret_rsq(g)
                if phase == "ret_fwd":
                    h2r_last = rpairs[NRT // 2 - 1][0]
                    dbg2 = fpool.tile([128, 256], FP, tag="dbg2")
                    nc.vector.tensor_copy(dbg2[:], h2r_last[:])
                    nc.sync.dma_start(out_d.ap()[0:128, 0:128],
                                      dbg2[:, 0:128])

    nc.compile()
    return nc


def _host_rho(inputs):
    """Per-token gradient weights rho_tok [b, 128, NT] (fp32)."""
    seq = np.asarray(inputs["seq"], np.float32)          # (b, n, d)
    b = seq.shape[0]
    reps = seq.reshape(b, NC, CHUNK, D)[:, :, 0]          # (b, nc, d)

    def sig(x):
        return 1.0 / (1.0 + np.exp(-x))

    lr = sig(reps @ np.asarray(inputs["w_lr"], np.float32)
             + np.asarray(inputs["b_lr"], np.float32))[..., 0]     # (b, nc)
    alpha = sig(reps @ np.asarray(inputs["w_decay"], np.float32)
                + np.asarray(inputs["b_decay"], np.float32))[..., 0]
    eta = sig(reps @ np.asarray(inputs["w_mom"], np.float32)
              + np.asarray(inputs["b_mom"], np.float32))[..., 0]
    keep = 1.0 - alpha

    K = np.ones((b, NC), np.float32)
    K[:, :-1] = np.cumprod(keep[:, ::-1], axis=1)[:, ::-1][:, 1:]
    Wm = np.empty((b, NC), np.float32)
    Wm[:, NC - 1] = 1.0
    for j in range(NC - 2, -1, -1):
        Wm[:, j] = K[:, j] + eta[:, j + 1] * Wm[:, j + 1]
    rho_chunk = (2.0 / D) * lr * Wm                       # (b, nc)

    rho_tok = np.empty((b, 128, NT), np.float32)
    for t in range(NT):
        rho_tok[:, 0:64, t] = rho_chunk[:, 2 * t, None]
        rho_tok[:, 64:128, t] = rho_chunk[:, 2 * t + 1, None]
    return rho_tok


def _prep_in_maps(inputs, nkeep):
    bf = ml_dtypes.bfloat16
    seq = np.ascontiguousarray(inputs["seq"], dtype=np.float32)
    gam = np.asarray(inputs["gamma0"], np.float32)
    assert np.allclose(gam, 1.0), "kernel assumes gamma0 == 1 (spec fill)"
    w1_0 = np.asarray(inputs["w1_0"], np.float32)
    w2 = np.asarray(inputs["w2_0"], dtype=np.float32)
    w2c = np.concatenate([w2[128 * c:128 * (c + 1), :] for c in range(4)],
                         axis=1)
    wkn = np.asarray(inputs["w_k"], np.float32)
    wvn = np.asarray(inputs["w_v"], np.float32)
    wqn = np.asarray(inputs["w_q"], np.float32)
    IDF = np.eye(128, dtype=np.float32)
    cpb = np.ascontiguousarray(np.concatenate(
        [wkn @ w1_0, wkn - wvn, w2c, w2.T, IDF, wkn @ wqn.T, wqn],
        axis=1)).astype(bf)
    assert cpb.shape[1] == _CPB_COLS

    rho_tok = _host_rho(inputs)
    seqb = seq.astype(bf)
    T0 = NT - nkeep
    W = nkeep * 128

    in_maps = []
    for c in range(NCORES):
        s, hf = divmod(c, 2)
        cpf = np.ascontiguousarray(
            np.concatenate([IDF, rho_tok[s]], axis=1), np.float32)
        assert cpf.shape[1] == _CPF_COLS
        suf = seqb[s, T0 * 128:]
        m = dict(
            cpb=cpb,
            cpf=cpf,
            seqT=np.ascontiguousarray(suf.T),
            seqG=np.ascontiguousarray(
                suf.reshape(nkeep, 128, D).transpose(1, 0, 2).reshape(
                    128, W)),
            seqrT=np.ascontiguousarray(seqb[s, hf * 2048:(hf + 1) * 2048].T),
        )
        in_maps.append(m)
    return in_maps


def _pick_nkeep(rho_tok, eps=1e-3):
    mx = np.abs(rho_tok).max(axis=1)          # (b, NT)
    gmax = float(mx.max())
    need = 1
    for s in range(mx.shape[0]):
        idx = np.where(mx[s] >= eps * gmax)[0]
        first = int(idx.min()) if idx.size else NT - 1
        need = max(need, NT - first)
    return min(NT, max(QT, ((need + QT - 1) // QT) * QT))


def _get_nc(nkeep=NT):
    key = f"nc{nkeep}"
    if key not in _CACHED:
        _CACHED[key] = build_nc(nkeep=nkeep)
    return _CACHED[key]


def kernel(**inputs) -> np.ndarray:
    nkeep = _pick_nkeep(_host_rho(inputs))
    nc = _get_nc(nkeep)
    in_maps = _prep_in_maps(inputs, nkeep)
    try:
        res = bass_utils.run_bass_kernel_spmd(nc, in_maps,
                                              core_ids=list(range(NCORES)))
    except Exception:
        res = bass_utils.run_bass_kernel_spmd(nc, in_maps,
                                              core_ids=list(range(NCORES)))
    out = np.empty((B, N, D), dtype=np.float32)
    for c in range(NCORES):
        s, hf = divmod(c, 2)
        out[s, hf * 2048:(hf + 1) * 2048] = res.results[c]["out"]
    return out


# revision 39
# speedup vs baseline: 1.5216x; 1.5216x over previous
"""Trainium2 Bass kernel for nn_NeuralMemory (test-time-training memory layer).

Mathematical reformulation (validated vs the jax reference):
  * Every per-chunk gradient is taken at the same initial params, and the
    two chunk-axis linear scans (momentum, decay) are linear in the
    gradients.  The final updated weights collapse to a single
    token-weighted backward pass with per-token weight
        rho_t = (2/d) * W_{c(t)} * lr_{c(t)},
        W_j   = K_j + eta_{j+1} W_{j+1},  K_j = prod_{i>j} keep_i.
    rho is computed on the HOST and shipped as a [128, 32] per-token-tile
    column table; the device never evaluates sigmoids or scans.
  * gamma0 is ones (spec fill), so the forward residual-norm scale drops
    out of the backward: with w := h*ri + (k-v),  rr := ri*rho,
        t3  = w*rr,   q1 = t3 .* h,   cp' = sum_f q1 = rr*cp
        s2  = -cp'*ri^2/D,   dh = h*s2 + t3
        dg  = sum_tok q1      (via PE transpose + DVE row-accum)
        dw1 = wk^T M,  M = sum_t S_t^T dz_t   (dw1 never materialized:
              retrieval uses w1c = -(wq wk^T) M with KT = wk wq^T host-made)
        dw2T += dh^T a
  * z is computed directly from seq via the host-folded w1k = wk @ w1_0
    in BOTH layouts (token-major pair-wide for a/gelu'(z), hidden-major
    pair-wide for the h matmul): no k projection/copy on device at all.
  * The per-token backward scalars (rr, rir, s3 with
    cp = ri*ss + cv, cv = sum_f kv.*h accumulated in fwd) are computed
    GROUP-BATCHED in the rsqrt step, so the per-tile critical chain is
    just kvr(Pool) -> dh = h*s3 + kvr (one DVE stt) -> dhT -> da -> dz
    -> M; the q1/dg path trails off-critical on Pool.
  * Retrieval z comes from seqrT via w1c (device-folded wq @ w1f); h/qt
    are evacuated from PSUM immediately (ACT bf16 / DVE fp32 copies) so
    the 2-slot PSUM rotation never gates the group pipeline.

Engine assignment targets balanced busy-time: wide [128,1024] 2-bank
PSUM tiles let one ACT instruction cover a token-tile PAIR (fixed
~185ns ACT overhead amortized); matmul cost scales with OUTPUT width
and PE p-states reward continuous streaks, so fwd emits pure PE/ACT
work and all DVE/Pool consumers (stats, bwd chains) are emitted where
they cannot poison another engine's in-order queue.  GPSIMD (Pool)
cannot touch PSUM, so it only gets SBUF elementwise work.

Sharding: core c handles sample s=c//2; grad replicated in the core
pair, retrieval split by half (no collectives -- a 2-core AllReduce
costs ~15us kickoff on this stack, more than the ~8us it would save).
"""
import numpy as np
import ml_dtypes

import concourse.bass as bass
import concourse.bacc as bacc
import concourse.mybir as mybir
import concourse.tile as tile
from concourse import bass_utils

FP = mybir.dt.float32
BF = mybir.dt.bfloat16
AF = mybir.ActivationFunctionType
OP = mybir.AluOpType

B, N, D, HID = 4, 4096, 128, 512
CHUNK = 64
NC = N // CHUNK            # 64 chunks
NT = N // 128              # 32 token-tiles (grad chain)
NRT = (N // 2) // 128      # 16 token-tiles (retrieval half)
QT = 4                     # grad tiles per group (rsqrt batching)
RQ = 4                     # retrieval tiles per group
NCORES = 8

_CACHED = {}

# cpb (bf16) column offsets
_W1K = 0           # wk @ w1_0, 512 cols
_WKV = 512         # wk - wv, 128
_W2C = 640         # w2 block layout [hid_c, (c,feat)], 512
_W2T = 1152        # w2^T [feat, hid], 512
_IDTB = 1664       # identity bf16, 128
_KT = 1792         # wk @ wq^T (for w1c = -(wq wk^T) M), 128
_WQ = 1920         # wq, 128
_CPB_COLS = 2048

# cpf (fp32) column offsets
_IDT = 0           # identity fp32
_RHO = 128         # rho_tok [128, 32]
_CPF_COLS = 160


def _emit_rsqrt(nc, wpool, ss, scale, bias, w, tagp):
    """ri = 1/sqrt(ss*scale + bias) on DVE only (Quake init + 1 Newton)."""
    I32 = mybir.dt.int32
    ms = wpool.tile([128, w], FP, tag=f"{tagp}q_ms")
    nc.vector.tensor_scalar(ms[:], ss, scale, bias, OP.mult, OP.add)
    qi = wpool.tile([128, w], I32, tag=f"{tagp}q_i")
    nc.vector.tensor_scalar(qi[:], ms[:].bitcast(I32), 1, None,
                            OP.arith_shift_right)
    qj = wpool.tile([128, w], I32, tag=f"{tagp}q_j")
    nc.vector.tensor_scalar(qj[:], qi[:], -1, 0x5F3759DF, OP.mult, OP.add)
    y = qj[:].bitcast(FP)
    a = wpool.tile([128, w], FP, tag=f"{tagp}q_a")
    nc.vector.tensor_mul(a[:], y, y)
    nc.vector.tensor_mul(a[:], a[:], ms[:])
    nc.vector.tensor_scalar(a[:], a[:], -0.5, 1.5, OP.mult, OP.add)
    yn = wpool.tile([128, w], FP, tag=f"{tagp}q_y")
    nc.vector.tensor_mul(yn[:], y, a[:])
    return yn[:]


def build_nc(repeat=1, nkeep=NT, das_pairs=None, phase="all"):
    nc = bacc.Bacc("TRN2", target_bir_lowering=False, debug=False)

    T0 = NT - nkeep
    W = nkeep * 128
    NP = nkeep // 2            # grad pairs
    NG = nkeep // QT           # grad groups
    if das_pairs is None:
        # das-route dz2 through ACT everywhere: ACT absorbs the psum
        # evacuation and DVE gets the cheap 2x SBUF multiply
        das_pairs = tuple(range(NP))

    # ---- DRAM I/O ----
    seqT_d = nc.dram_tensor("seqT", [D, W], BF, kind="ExternalInput")
    seqG_d = nc.dram_tensor("seqG", [128, W], BF, kind="ExternalInput")
    seqrT_d = nc.dram_tensor("seqrT", [D, N // 2], BF, kind="ExternalInput")
    cpb_d = nc.dram_tensor("cpb", [128, _CPB_COLS], BF, kind="ExternalInput")
    cpf_d = nc.dram_tensor("cpf", [128, _CPF_COLS], FP, kind="ExternalInput")
    out_d = nc.dram_tensor("out", [N // 2, D], FP, kind="ExternalOutput")

    with tile.TileContext(nc) as tc:
        with (
            tc.tile_pool(name="const", bufs=1) as cpool,
            tc.tile_pool(name="seq", bufs=1) as spool,
            tc.tile_pool(name="fin", bufs=2) as fpool,
            tc.tile_pool(name="work", bufs=4) as wpool,
            tc.tile_pool(name="qcol", bufs=3) as qpool,
            tc.tile_pool(name="pacc", bufs=1, space="PSUM") as pacc,
            tc.tile_pool(name="pwide", bufs=2, space="PSUM") as pwide,
            tc.tile_pool(name="pmid", bufs=2, space="PSUM") as pmid,
        ):
            # ---- constants & sequence into SBUF: separate tiles per DMA
            # chunk so consumers only wait on their own transfer ----
            cpbA = cpool.tile([128, 640], BF)    # w1k | wkv
            cpbB = cpool.tile([128, 1152], BF)   # w2c | w2T | IDTb
            cpbC = cpool.tile([128, 256], BF)    # KT | wq
            cpf = cpool.tile([128, _CPF_COLS], FP)
            seqTs = [spool.tile([D, 256], BF, name=f"seqT{p}")
                     for p in range(NP)]
            seqGs = [spool.tile([128, 512], BF, name=f"seqG{g}")
                     for g in range(NG)]
            seqrT = spool.tile([D, N // 2], BF)

            nc.sync.dma_start(cpbA[:], cpb_d.ap()[:, 0:640])
            nc.sync.dma_start(seqTs[0][:], seqT_d.ap()[:, 0:256])
            nc.sync.dma_start(cpbB[:], cpb_d.ap()[:, 640:1792])
            nc.sync.dma_start(cpf[:], cpf_d.ap())
            for p in range(1, NP):
                nc.sync.dma_start(seqTs[p][:],
                                  seqT_d.ap()[:, p * 256:(p + 1) * 256])
            for g in range(NG):
                nc.sync.dma_start(seqGs[g][:],
                                  seqG_d.ap()[:, g * 512:(g + 1) * 512])
            nc.sync.dma_start(cpbC[:], cpb_d.ap()[:, 1792:2048])
            for j in range(2):
                nc.sync.dma_start(seqrT[:, j * 1024:(j + 1) * 1024],
                                  seqrT_d.ap()[:, j * 1024:(j + 1) * 1024])

            w1k = cpbA[:, 0:512]
            wkv = cpbA[:, 512:640]
            w2c = cpbB[:, 0:512]
            w2T = cpbB[:, 512:1024]
            IDTb = cpbB[:, 1024:1152]
            KT = cpbC[:, 0:128]
            wq = cpbC[:, 128:256]
            IDT = cpf[:, _IDT:_IDT + 128]
            rho = cpf[:, _RHO:_RHO + NT]

            def seqT_pair(p):
                return seqTs[p][:]

            def seqT_tile(p, j):
                return seqTs[p][:, j * 128:(j + 1) * 128]

            for _rep in range(repeat):
                # =========================================================
                # Gradient chain over the kept suffix: software-pipelined
                # groups of QT tiles; M/dw2T accumulated in PSUM.
                # =========================================================
                M_acc = pacc.tile([D, HID], FP, tag="M")
                W2_acc = pacc.tile([128, HID], FP, tag="W2")
                dgparts = fpool.tile([128, NP], FP, tag="dgparts")

                pairs = {}      # p -> sbuf tiles from fwd
                quads = {}      # q -> ss4 or batched scalar columns
                cv4s = {}       # q -> cv accumulator [128, QT]
                dz2s = {}       # p -> dz2 tile (for deferred M matmuls)

                def grad_fwd_pair(p):
                    t0 = 2 * p
                    q, j0 = divmod(t0, QT)
                    if j0 == 0:
                        quads[q] = qpool.tile([128, QT], FP, tag="ss4",
                                              name="ss4")
                        cv4s[q] = qpool.tile([128, QT], FP, tag="cv4",
                                             name="cv4")

                    # hidden-major z pair first: afab -> mix -> h2kv is the
                    # chain gating the group's rsq, so it leads ACT's queue
                    zf = pwide.tile([128, 1024], FP, tag="w", name="zf")
                    for c in range(4):
                        nc.tensor.matmul(zf[:, c * 256:(c + 1) * 256],
                                         w1k[:, c * 128:(c + 1) * 128],
                                         seqT_pair(p), start=True,
                                         stop=True)
                    afab = wpool.tile([128, 1024], BF, tag="afab", bufs=3)
                    nc.scalar.activation(afab[:], zf[:], AF.Gelu)

                    # h(t0)|h(t1)|kv(t0)|kv(t1) in one bank
                    mix = pmid.tile([128, 512], FP, tag="m", name="mix")
                    for j in range(2):
                        for c in range(4):
                            nc.tensor.matmul(
                                mix[:, j * 128:(j + 1) * 128],
                                afab[:, c * 256 + j * 128:
                                     c * 256 + (j + 1) * 128],
                                w2c[:, c * 128:(c + 1) * 128],
                                start=(c == 0), stop=(c == 3))
                        nc.tensor.matmul(mix[:, 256 + j * 128:
                                             256 + (j + 1) * 128],
                                         seqT_tile(p, j),
                                         wkv, start=True, stop=True)
                    # single pair-batched PSUM->SBUF copy: ACT normally,
                    # DVE for the group-last pair (its copy gates the
                    # group rsqrt and would otherwise sit behind the
                    # pair's gelus in ACT's in-order queue)
                    h2kv = wpool.tile([128, 512], BF, tag="h2kv", bufs=6)
                    if t0 % QT == QT - 2:
                        nc.vector.tensor_copy(h2kv[:], mix[:])
                    else:
                        nc.scalar.copy(h2kv[:], mix[:])
                    # token-major z pair (one wide 2-bank psum tile)
                    zp = pwide.tile([128, 1024], FP, tag="w", name="zp")
                    for j in range(2):
                        nc.tensor.matmul(zp[:, j * 512:(j + 1) * 512],
                                         seqT_tile(p, j),
                                         w1k, start=True, stop=True)
                    a_tm2 = wpool.tile([128, 1024], BF, tag="a_tm2", bufs=4)
                    nc.scalar.activation(a_tm2[:], zp[:], AF.Gelu)
                    gp2 = wpool.tile([128, 1024], BF, tag="gp2", bufs=4)
                    nc.scalar.activation(gp2[:], zp[:], AF.Derivative_Gelu)
                    pairs[p] = (h2kv, a_tm2, gp2)

                def grad_stats_pair(p):
                    # ss/cv accumulation, off the critical bwd chain
                    t0 = 2 * p
                    q, j0 = divmod(t0, QT)
                    h2kv, a_tm2, gp2 = pairs[p]
                    scr = wpool.tile([128, 256], BF, tag="scr", bufs=6)
                    nc.gpsimd.tensor_mul(scr[:], h2kv[:, 0:256],
                                         h2kv[:, 0:256])
                    sdead = wpool.tile([128, 128], BF, tag="sdead", bufs=4)
                    cvscr = wpool.tile([128, 128], BF, tag="cvscr", bufs=4)
                    for j in range(2):
                        nc.vector.tensor_scalar(
                            sdead[:], scr[:, j * 128:(j + 1) * 128],
                            1.0, 0.0, OP.mult, OP.add,
                            accum_out=quads[q][:, j0 + j:j0 + j + 1])
                        nc.vector.scalar_tensor_tensor(
                            cvscr[:], h2kv[:, 256 + j * 128:
                                           256 + (j + 1) * 128], 1.0,
                            h2kv[:, j * 128:(j + 1) * 128], OP.mult, OP.mult,
                            accum_out=cv4s[q][:, j0 + j:j0 + j + 1])

                def grad_rsq(q):
                    ss4 = quads[q]
                    cv4 = cv4s[q]
                    ri4 = _emit_rsqrt(nc, qpool, ss4[:], 1.0 / D, 1e-6,
                                      QT, "g")
                    rr4 = qpool.tile([128, QT], FP, tag="rr4")
                    nc.vector.tensor_mul(rr4[:], ri4,
                                         rho[:, T0 + QT * q:
                                              T0 + QT * q + QT])
                    rir = qpool.tile([128, QT], FP, tag="rir")
                    nc.vector.tensor_mul(rir[:], ri4, rr4[:])
                    # s3 = rir + s2,  s2 = -(rr*ri^2/D) * (ri*ss + cv)
                    e2 = qpool.tile([128, QT], FP, tag="e2")
                    nc.vector.tensor_mul(e2[:], ri4, ss4[:])
                    nc.vector.tensor_add(e2[:], e2[:], cv4[:])
                    f1 = qpool.tile([128, QT], FP, tag="f1")
                    nc.vector.tensor_mul(f1[:], ri4, ri4)
                    nc.vector.tensor_mul(f1[:], f1[:], rr4[:])
                    nc.vector.tensor_mul(f1[:], f1[:], e2[:])
                    s3 = qpool.tile([128, QT], FP, tag="s3")
                    nc.vector.scalar_tensor_tensor(s3[:], f1[:], -1.0 / D,
                                                   rir[:], OP.mult, OP.add)
                    quads[q] = (rr4, rir, s3)

                def grad_bwd_pair(p):
                    t0 = 2 * p
                    q = t0 // QT
                    rr4, rir, s3 = quads[q]
                    h2kv, a_tm2, gp2 = pairs.pop(p)
                    da2 = pwide.tile([128, 1024], FP, tag="w", name="da2")
                    tr = pmid.tile([128, 512], BF, tag="m", name="tr")
                    cols = [(rr4[:, (t0 + j) % QT:(t0 + j) % QT + 1],
                             rir[:, (t0 + j) % QT:(t0 + j) % QT + 1],
                             s3[:, (t0 + j) % QT:(t0 + j) % QT + 1],
                             h2kv[:, j * 128:(j + 1) * 128],
                             h2kv[:, 256 + j * 128:256 + (j + 1) * 128])
                            for j in range(2)]
                    # critical chain for BOTH tiles first:
                    # dh = h*s3 + kv*rr -> dhT -> da -> dz -> M
                    kvrs, dhs = [], []
                    for j, (rr_c, rir_c, s3_c, h_sb, kv) in enumerate(cols):
                        kvr = wpool.tile([128, 128], BF, tag="kvr", bufs=8)
                        nc.gpsimd.tensor_scalar_mul(kvr[:], kv, rr_c)
                        kvrs.append(kvr)
                    for j, (rr_c, rir_c, s3_c, h_sb, kv) in enumerate(cols):
                        dh = wpool.tile([128, 128], BF, tag="dh", bufs=8)
                        nc.vector.scalar_tensor_tensor(dh[:], h_sb, s3_c,
                                                       kvrs[j][:], OP.mult,
                                                       OP.add)
                        dhs.append(dh)
                        nc.tensor.transpose(tr[:, j * 128:(j + 1) * 128],
                                            dh[:], IDTb)
                    dhT2 = wpool.tile([128, 256], BF, tag="dhT2", bufs=6)
                    nc.vector.tensor_copy(dhT2[:], tr[:, 0:256])
                    for j in range(2):
                        nc.tensor.matmul(da2[:, j * 512:(j + 1) * 512],
                                         dhT2[:, j * 128:(j + 1) * 128],
                                         w2T, start=True, stop=True)
                    dz2 = wpool.tile([128, 1024], BF, tag="dz2", bufs=3)
                    if p in das_pairs:
                        das2 = wpool.tile([128, 1024], BF, tag="das2",
                                          bufs=3)
                        nc.scalar.copy(das2[:], da2[:])
                        nc.vector.tensor_mul(dz2[:], das2[:], gp2[:])
                    else:
                        nc.vector.tensor_mul(dz2[:], da2[:], gp2[:])
                    dz2s[p] = dz2
                    # off-critical trail: W2 accumulation, dg path
                    for j, (rr_c, rir_c, s3_c, h_sb, kv) in enumerate(cols):
                        t = t0 + j
                        nc.tensor.matmul(W2_acc[:], dhs[j][:],
                                         a_tm2[:, j * 512:(j + 1) * 512],
                                         start=(t == 0), stop=(t == nkeep - 1))
                        u1 = wpool.tile([128, 128], BF, tag="u1", bufs=8)
                        nc.gpsimd.tensor_scalar_mul(u1[:], h_sb, rir_c)
                        t3 = wpool.tile([128, 128], BF, tag="t3", bufs=8)
                        nc.gpsimd.tensor_add(t3[:], u1[:], kvrs[j][:])
                        q1 = wpool.tile([128, 128], BF, tag="q1", bufs=8)
                        nc.gpsimd.tensor_mul(q1[:], t3[:], h_sb)
                        nc.tensor.transpose(tr[:, 256 + j * 128:
                                               256 + (j + 1) * 128],
                                            q1[:], IDTb)
                    gdead = wpool.tile([128, 256], BF, tag="gdead", bufs=4)
                    nc.vector.tensor_scalar(
                        gdead[:], tr[:, 256:512],
                        1.0, 0.0, OP.mult, OP.add,
                        accum_out=dgparts[:, p:p + 1])

                def grad_m_pair(p):
                    # deferred M matmuls (wait on the slow dz2 DVE op, so
                    # emitted after the next fwd's PE work)
                    t0 = 2 * p
                    dz2 = dz2s.pop(p)
                    for j in range(2):
                        t = t0 + j
                        nc.tensor.matmul(M_acc[:],
                                         seqGs[p // 2][:, (t0 % 4 + j) * 128:
                                                       (t0 % 4 + j + 1) * 128],
                                         dz2[:, j * 512:(j + 1) * 512],
                                         start=(t == 0), stop=(t == nkeep - 1))

                # software pipeline: fwd(q) feeds ACT/PE with ready work
                # (no DVE/Pool ops inside), bwd(q-1) critical chains run
                # on DVE/Pool behind it, deferred M and stats last
                for q in range(NG + 1):
                    for pj in range(QT // 2):
                        if q < NG:
                            grad_fwd_pair((QT * q) // 2 + pj)
                        if q >= 1:
                            grad_bwd_pair((QT * (q - 1)) // 2 + pj)
                            if q == NG:
                                # tail: PE is idle, don't defer M
                                grad_m_pair((QT * (q - 1)) // 2 + pj)
                        if q < NG:
                            grad_stats_pair((QT * q) // 2 + pj)
                    if q < NG:
                        grad_rsq(q)
                    if 1 <= q < NG:
                        for pj in range(QT // 2):
                            grad_m_pair((QT * (q - 1)) // 2 + pj)

                # =========================================================
                # Bridge: final weights for retrieval
                #   w1c = -(wq wk^T) M,  w2f = -(dw2T)^T,  gfb = -dg bcast
                # =========================================================
                Msb = fpool.tile([D, HID], BF, tag="Msb")
                nc.scalar.copy(Msb[:], M_acc[:])
                w2Tf = fpool.tile([128, HID], BF, tag="w2Tf")
                nc.scalar.activation(w2Tf[:], W2_acc[:], AF.Copy, scale=-1.0)

                ps_w1c = pwide.tile([128, 512], FP, tag="w", name="ps_w1c")
                nc.tensor.matmul(ps_w1c[:], KT, Msb[:], start=True, stop=True)
                w1c = fpool.tile([D, HID], BF, tag="w1c")
                nc.scalar.activation(w1c[:], ps_w1c[:], AF.Copy, scale=-1.0)

                ps_w2 = pmid.tile([128, 512], BF, tag="m", name="ps_w2")
                for c in range(4):
                    nc.tensor.transpose(ps_w2[:, c * 128:(c + 1) * 128],
                                        w2Tf[:, c * 128:(c + 1) * 128], IDTb)
                w2f = fpool.tile([128, HID], BF, tag="w2f")
                nc.vector.tensor_copy(w2f[:], ps_w2[:])

                # gf = -sum_p dgparts ; gfb2 = broadcast row, twice
                gfc = fpool.tile([128, 1], FP, tag="gfc")
                gdead2 = fpool.tile([128, NP], FP, tag="gdead2")
                nc.vector.tensor_scalar(gdead2[:], dgparts[:], -1.0, 0.0,
                                        OP.mult, OP.add, accum_out=gfc[:])
                ps_gT = pmid.tile([1, 128], FP, tag="m", name="ps_gT")
                nc.tensor.transpose(ps_gT[:], gfc[:], IDT)
                gT = fpool.tile([1, 128], FP, tag="gT")
                nc.scalar.copy(gT[:], ps_gT[:])
                ones_r = fpool.tile([1, 128], FP, tag="ones_r")
                nc.vector.memset(ones_r[:], 1.0)
                ps_gfb = pwide.tile([128, 128], FP, tag="w", name="ps_gfb")
                nc.tensor.matmul(ps_gfb[:], ones_r[:], gT[:],
                                 start=True, stop=True)
                gfb2 = fpool.tile([128, 256], BF, tag="gfb2")
                nc.vector.tensor_copy(gfb2[:, 0:128], ps_gfb[:])
                nc.vector.tensor_copy(gfb2[:, 128:256], gfb2[:, 0:128])

                if phase == "grad":
                    # debug: dump bridge results and skip retrieval
                    dbg = fpool.tile([128, 256], FP, tag="dbg")
                    nc.vector.tensor_copy(dbg[:, 0:4], dgparts[:])
                    nc.vector.tensor_copy(dbg[:, 4:5], gfc[:])
                    nc.sync.dma_start(out_d.ap()[0:128, 0:128],
                                      dbg[:, 0:128])
                    continue

                # =========================================================
                # Retrieval on this core's half (16 tiles, pipelined)
                # =========================================================
                rpairs = {}
                rquads = {}
                opacks = {}

                def ret_fwd_pair(r):
                    i0 = 2 * r
                    g, j0 = divmod(i0, RQ)
                    if j0 == 0:
                        rquads[g] = qpool.tile([128, RQ], FP, tag="ss2",
                                               name="ss2")
                    ss2 = rquads[g]
                    Sr2 = seqrT[:, i0 * 128:(i0 + 2) * 128]

                    z2 = pwide.tile([128, 1024], FP, tag="w", name="z2")
                    for c in range(4):
                        nc.tensor.matmul(z2[:, c * 256:(c + 1) * 256],
                                         w1c[:, c * 128:(c + 1) * 128],
                                         Sr2, start=True, stop=True)
                    a2 = wpool.tile([128, 1024], BF, tag="a2", bufs=3)
                    nc.scalar.activation(a2[:], z2[:], AF.Gelu)

                    hq = pmid.tile([128, 512], FP, tag="m", name="hq")
                    for j in range(2):
                        for c in range(4):
                            nc.tensor.matmul(
                                hq[:, j * 128:(j + 1) * 128],
                                a2[:, c * 256 + j * 128:
                                   c * 256 + (j + 1) * 128],
                                w2f[:, c * 128:(c + 1) * 128],
                                start=(c == 0), stop=(c == 3))
                        nc.tensor.matmul(hq[:, 256 + j * 128:
                                             256 + (j + 1) * 128],
                                         seqrT[:, (i0 + j) * 128:
                                               (i0 + j + 1) * 128],
                                         wq, start=True, stop=True)
                    # evacuate PSUM immediately: h2 (bf16, ACT), qt (fp32,
                    # DVE); ss via Pool square + DVE row-accum from SBUF
                    h2r = wpool.tile([128, 256], BF, tag="h2r", bufs=6)
                    nc.scalar.copy(h2r[:], hq[:, 0:256])
                    qtr = wpool.tile([128, 256], FP, tag="qtr", bufs=6)
                    nc.vector.tensor_copy(qtr[:], hq[:, 256:512])
                    scr2 = wpool.tile([128, 256], BF, tag="scr2", bufs=6)
                    sdead2 = wpool.tile([128, 128], BF, tag="sdead2", bufs=4)
                    if g == NRT // RQ - 1:  # noqa: scr2 unused here
                        # last group: ss on ACT (idle in the tail) so the
                        # final rsqrt isn't stuck behind DVE's out backlog
                        for j in range(2):
                            nc.scalar.activation(
                                sdead2[:], h2r[:, j * 128:(j + 1) * 128],
                                AF.Square,
                                accum_out=ss2[:, j0 + j:j0 + j + 1])
                    else:
                        scr2 = wpool.tile([128, 256], BF, tag="scr2",
                                          bufs=6)
                        nc.gpsimd.tensor_mul(scr2[:], h2r[:], h2r[:])
                        for j in range(2):
                            nc.vector.tensor_scalar(
                                sdead2[:], scr2[:, j * 128:(j + 1) * 128],
                                1.0, 0.0, OP.mult, OP.add,
                                accum_out=ss2[:, j0 + j:j0 + j + 1])
                    rpairs[r] = (h2r, qtr)

                def ret_rsq(g):
                    r2i = _emit_rsqrt(nc, qpool, rquads[g][:], 1.0 / D,
                                      1e-6, RQ, "r")
                    rquads[g] = r2i

                def ret_out_pair(r):
                    i0 = 2 * r
                    g = i0 // RQ
                    r2i = rquads[g]
                    h2r, qtr = rpairs.pop(r)
                    hn2p = wpool.tile([128, 256], BF, tag="hn2p", bufs=6)
                    for j in range(2):
                        jj = (i0 + j) % RQ
                        nc.vector.tensor_scalar(
                            hn2p[:, j * 128:(j + 1) * 128],
                            h2r[:, j * 128:(j + 1) * 128],
                            r2i[:, jj:jj + 1], None, OP.mult)
                    o1p = wpool.tile([128, 256], BF, tag="o1p", bufs=6)
                    nc.vector.tensor_mul(o1p[:], hn2p[:], gfb2[:])
                    if r % 2 == 0:
                        opacks[g] = wpool.tile([128, 512], FP, tag="opack",
                                               name="opack", bufs=3)
                    opack = opacks[g]
                    off = (r % 2) * 256
                    if g == NRT // RQ - 1:
                        # last group: add on DVE (327ns vs Pool's 603)
                        nc.vector.tensor_add(opack[:, off:off + 256],
                                             o1p[:], qtr[:])
                    else:
                        nc.gpsimd.tensor_add(opack[:, off:off + 256],
                                             o1p[:], qtr[:])
                    if r % 2 == 1:
                        dst = out_d.ap()[g * 512:(g + 1) * 512,
                                         :].rearrange("(j p) d -> p j d",
                                                      p=128)
                        nc.sync.dma_start(
                            dst, opack[:].rearrange("p (j d) -> p j d",
                                                    d=128))

                for g in range(NRT // RQ + 1):
                    for rj in range(RQ // 2):
                        if g < NRT // RQ:
                            ret_fwd_pair((RQ * g) // 2 + rj)
                        if g >= 1 and phase != "ret_fwd":
                            ret_out_pair((RQ * (g - 1)) // 2 + rj)
                    if g < NRT // RQ:
                        ret_rsq(g)
                if phase == "ret_fwd":
                    h2r_last = rpairs[NRT // 2 - 1][0]
                    dbg2 = fpool.tile([128, 256], FP, tag="dbg2")
                    nc.vector.tensor_copy(dbg2[:], h2r_last[:])
                    nc.sync.dma_start(out_d.ap()[0:128, 0:128],
                                      dbg2[:, 0:128])

    nc.compile()
    return nc


def _host_rho(inputs):
    """Per-token gradient weights rho_tok [b, 128, NT] (fp32)."""
    seq = np.asarray(inputs["seq"], np.float32)          # (b, n, d)
    b = seq.shape[0]
    reps = seq.reshape(b, NC, CHUNK, D)[:, :, 0]          # (b, nc, d)

    def sig(x):
        return 1.0 / (1.0 + np.exp(-x))

    lr = sig(reps @ np.asarray(inputs["w_lr"], np.float32)
             + np.asarray(inputs["b_lr"], np.float32))[..., 0]     # (b, nc)
    alpha = sig(reps @ np.asarray(inputs["w_decay"], np.float32)
                + np.asarray(inputs["b_decay"], np.float32))[..., 0]
    eta = sig(reps @ np.asarray(inputs["w_mom"], np.float32)
              + np.asarray(inputs["b_mom"], np.float32))[..., 0]
    keep = 1.0 - alpha

    K = np.ones((b, NC), np.float32)
    K[:, :-1] = np.cumprod(keep[:, ::-1], axis=1)[:, ::-1][:, 1:]
    Wm = np.empty((b, NC), np.float32)
    Wm[:, NC - 1] = 1.0
    for j in range(NC - 2, -1, -1):
        Wm[:, j] = K[:, j] + eta[:, j + 1] * Wm[:, j + 1]
    rho_chunk = (2.0 / D) * lr * Wm                       # (b, nc)

    rho_tok = np.empty((b, 128, NT), np.float32)
    for t in range(NT):
        rho_tok[:, 0:64, t] = rho_chunk[:, 2 * t, None]
        rho_tok[:, 64:128, t] = rho_chunk[:, 2 * t + 1, None]
    return rho_tok


def _prep_in_maps(inputs, nkeep):
    bf = ml_dtypes.bfloat16
    seq = np.ascontiguousarray(inputs["seq"], dtype=np.float32)
    gam = np.asarray(inputs["gamma0"], np.float32)
    assert np.allclose(gam, 1.0), "kernel assumes gamma0 == 1 (spec fill)"
    w1_0 = np.asarray(inputs["w1_0"], np.float32)
    w2 = np.asarray(inputs["w2_0"], dtype=np.float32)
    w2c = np.concatenate([w2[128 * c:128 * (c + 1), :] for c in range(4)],
                         axis=1)
    wkn = np.asarray(inputs["w_k"], np.float32)
    wvn = np.asarray(inputs["w_v"], np.float32)
    wqn = np.asarray(inputs["w_q"], np.float32)
    IDF = np.eye(128, dtype=np.float32)
    cpb = np.ascontiguousarray(np.concatenate(
        [wkn @ w1_0, wkn - wvn, w2c, w2.T, IDF, wkn @ wqn.T, wqn],
        axis=1)).astype(bf)
    assert cpb.shape[1] == _CPB_COLS

    rho_tok = _host_rho(inputs)
    seqb = seq.astype(bf)
    T0 = NT - nkeep
    W = nkeep * 128

    in_maps = []
    for c in range(NCORES):
        s, hf = divmod(c, 2)
        cpf = np.ascontiguousarray(
            np.concatenate([IDF, rho_tok[s]], axis=1), np.float32)
        assert cpf.shape[1] == _CPF_COLS
        suf = seqb[s, T0 * 128:]
        m = dict(
            cpb=cpb,
            cpf=cpf,
            seqT=np.ascontiguousarray(suf.T),
            seqG=np.ascontiguousarray(
                suf.reshape(nkeep, 128, D).transpose(1, 0, 2).reshape(
                    128, W)),
            seqrT=np.ascontiguousarray(seqb[s, hf * 2048:(hf + 1) * 2048].T),
        )
        in_maps.append(m)
    return in_maps


def _pick_nkeep(rho_tok, eps=1e-3):
    mx = np.abs(rho_tok).max(axis=1)          # (b, NT)
    gmax = float(mx.max())
    need = 1
    for s in range(mx.shape[0]):
        idx = np.where(mx[s] >= eps * gmax)[0]
        first = int(idx.min()) if idx.size else NT - 1
        need = max(need, NT - first)
    return min(NT, max(QT, ((need + QT - 1) // QT) * QT))


def _get_nc(nkeep=NT):
    key = f"nc{nkeep}"
    if key not in _CACHED:
        _CACHED[key] = build_nc(nkeep=nkeep)
    return _CACHED[key]


def kernel(**inputs) -> np.ndarray:
    nkeep = _pick_nkeep(_host_rho(inputs))
    nc = _get_nc(nkeep)
    in_maps = _prep_in_maps(inputs, nkeep)
    try:
        res = bass_utils.run_bass_kernel_spmd(nc, in_maps,
                                              core_ids=list(range(NCORES)))
    except Exception:
        res = bass_utils.run_bass_kernel_spmd(nc, in_maps,
                                              core_ids=list(range(NCORES)))
    out = np.empty((B, N, D), dtype=np.float32)
    for c in range(NCORES):
        s, hf = divmod(c, 2)
        out[s, hf * 2048:(hf + 1) * 2048] = res.results[c]["out"]
    return out
